# revision 28
# baseline (speedup 1.0000x reference)
"""Multihead attention on 8 Trainium2 cores (Bass/Tile).

Sharding: core = (batch b, head-group hg); 2 batches x 4 head-groups,
4 heads per core (head dim 64, local width 256).

Per core (matmul operands bf16, PSUM accumulation fp32):
  qT = (Wq[hg]/8 @ x_q^T)          [256, 2048]   (e' on partitions)
  kT = (Wk[hg]  @ x_k^T)           [256, SK]
  v  = (x_k  @ Wv[hg]^T)           [SK, 256]     (+ ones col -> denominator)
  scoresT[sk, sq] = kT^T-slices x qT  (PE, per k-tile, both head-halves)
  probsT = exp(scoresT)            (ACT, PSUM fp32 -> SBUF bf16)
  attnT[d, sq], denom[sq] = v_aug^T @ probsT
  attn = attnT * recip(denom)      (DVE recip on the PSUM denom row,
                                    DRAM-bounce broadcast, DVE muls)
  out_partial[s, :] = attn^T-chunks x Wo[:, hg]^T   (no bias on device)
Host: out[b] = sum of the 4 head-group partials + bo.

Schedule: lag-1 software pipeline over 8 stages (4 q-groups x 2
partition-chunks). Stage s emits scores(s)+exp(s) per k-tile with the
scores PSUM double-buffered via tag rotation, attnV(s-1) interleaved
per k-tile, norm(s-1) at stage end, and out-projection tiles filled
2/stage. The v-projection runs inside stage 1. The drain c-splits the
last out-proj tiles around the final norm chain so the PE never idles.

DMA: all host tensors are pre-arranged so every SBUF partition row is
one contiguous descriptor (128 descriptors per chunk DMA); wq/xq
chunks are issued first and gate the first projection matmuls ~2us in.

Mask handling: the key_padding_mask (and compaction padding) zeroes v
rows AND the ones-column, so masked keys contribute exactly 0 to both
the numerator and the softmax denominator.

Compaction: only valid (unmasked) key positions are shipped per batch,
padded to a multiple of 128. Kernels are compiled once per SK.
"""

import os
import sys

sys.path.insert(0, "/opt/trn_rl_repo")

import ml_dtypes
import numpy as np

import concourse.bass as bass
import concourse.mybir as mybir
import concourse.tile as tile
from concourse import bacc
from concourse.bass_utils import run_bass_kernel_spmd

B, S, E, H, D = 2, 2048, 1024, 16, 64
N_CORES = 8
HL = H // 4          # 4 heads per core
EL = HL * D          # 256 local embed width
PC = EL // 128       # 2 partition chunks of local heads
ECH = E // 128       # 8 contraction chunks for projections
SGRP = S // 512      # 4 query groups
SQT = S // 128       # 16 query tiles

f32 = mybir.dt.float32
bf16 = mybir.dt.bfloat16
nbf16 = ml_dtypes.bfloat16

_cache: dict[int, object] = {}
LAST_EXEC_NS = None
LAST_RESULTS = None


def _bcast_ap(handle, shape):
    """DRAM AP broadcast along partitions: shape [128, ...dims of handle]."""
    ap = handle[:]
    dims = [[0, shape[0]]]
    sizes = shape[1:]
    stride = 1
    rev = []
    for s in reversed(sizes):
        rev.append([stride, s])
        stride *= s
    dims.extend(reversed(rev))
    return bass.AP(tensor=ap.tensor, offset=0, ap=dims)


def _build(SK: int):
    SKT = SK // 128
    kgroups = [(o, min(512, SK - o)) for o in range(0, SK, 512)]

    nc = bacc.Bacc(None, target_bir_lowering=False)

    # all host-side pre-arranged: partition-major, contiguous per partition
    xqD = nc.dram_tensor("xqD", [128, ECH, S], bf16, kind="ExternalInput")
    xkD = nc.dram_tensor("xkD", [128, ECH, SK], bf16, kind="ExternalInput")
    # xv is k-tile-major so v-proj passes gate on per-tile chunks
    xvD = nc.dram_tensor("xvD", [128, SKT, ECH, 128], bf16, kind="ExternalInput")
    wqD = nc.dram_tensor("wqD", [128, ECH, EL], bf16, kind="ExternalInput")
    wkD = nc.dram_tensor("wkD", [128, ECH, EL], bf16, kind="ExternalInput")
    wvD = nc.dram_tensor("wvD", [128, ECH, EL], bf16, kind="ExternalInput")
    woD = nc.dram_tensor("woD", [128, PC, E], bf16, kind="ExternalInput")
    bqD = nc.dram_tensor("bqD", [128, PC], f32, kind="ExternalInput")
    bkD = nc.dram_tensor("bkD", [128, PC], f32, kind="ExternalInput")
    bvD = nc.dram_tensor("bvD", [HL * 65], f32, kind="ExternalInput")
    mkD = nc.dram_tensor("mkD", [128, SKT], f32, kind="ExternalInput")
    out = nc.dram_tensor("out", [S, E], bf16, kind="ExternalOutput")

    with tile.TileContext(nc) as tc, nc.allow_low_precision("bf16 attention"):
        with (
            tc.tile_pool(name="persist", bufs=1) as pp,
            tc.tile_pool(name="prb", bufs=2 * SKT + 2) as prb,
            tc.tile_pool(name="rcp", bufs=2) as rcp,
            tc.tile_pool(name="bsb", bufs=2) as bsb,
            tc.tile_pool(name="tmp", bufs=2) as tmp,
            tc.tile_pool(name="vtp", bufs=2) as vtp,
            tc.tile_pool(name="osb", bufs=3) as osb,
        ):
            # ---- persistent tiles ----
            wq_sb = pp.tile([128, ECH, EL], bf16, name="wq_sb", tag="wq_sb")
            wk_sb = pp.tile([128, ECH, EL], bf16, name="wk_sb", tag="wk_sb")
            wv_sb = pp.tile([128, ECH, EL], bf16, name="wv_sb", tag="wv_sb")
            wo_sb = pp.tile([128, PC, E], bf16, name="wo_sb", tag="wo_sb")
            bq_sb = pp.tile([128, PC], f32, name="bq_sb", tag="bq_sb")
            bk_sb = pp.tile([128, PC], f32, name="bk_sb", tag="bk_sb")
            bv_sb = pp.tile([128, HL, 65], f32, name="bv_sb", tag="bv_sb")
            m_sb = pp.tile([128, SKT], f32, name="m_sb", tag="m_sb")
            xq_sb = pp.tile([128, ECH, S], bf16, name="xq_sb", tag="xq_sb")
            xk_sb = pp.tile([128, ECH, SK], bf16, name="xk_sb", tag="xk_sb")
            xv_sb = pp.tile([128, SKT, ECH, 128], bf16, name="xv_sb", tag="xv_sb")

            qT_sb = [pp.tile([128, S], bf16, name=f"qT{c}", tag=f"qT{c}") for c in range(PC)]
            kT_sb = [pp.tile([128, SK], bf16, name=f"kT{c}", tag=f"kT{c}") for c in range(PC)]
            aT_sb = [pp.tile([128, S], bf16, name=f"aT{c}", tag=f"aT{c}") for c in range(PC)]
            v_sb = [
                pp.tile([128, HL, 65], bf16, name=f"v{t}", tag=f"v{t}") for t in range(SKT)
            ]

            # ---- DMA issue order == consumption order; each dma_start
            # costs ~0.7us serial on the SP sequencer, so keep the count low
            # while still chunking enough to gate the projection pipeline.
            nc.sync.dma_start(out=wq_sb, in_=wqD[:, :, :])
            nc.sync.dma_start(out=xq_sb[:, 0, :], in_=xqD[:, 0, :])
            nc.sync.dma_start(out=xq_sb[:, 1, :], in_=xqD[:, 1, :])
            nc.sync.dma_start(out=xq_sb[:, 2:4, :], in_=xqD[:, 2:4, :])
            nc.sync.dma_start(out=bq_sb, in_=bqD[:, :])
            nc.sync.dma_start(out=bk_sb, in_=bkD[:, :])
            nc.sync.dma_start(out=xq_sb[:, 4:8, :], in_=xqD[:, 4:8, :])
            nc.sync.dma_start(out=wk_sb, in_=wkD[:, :, :])
            nc.sync.dma_start(out=xk_sb[:, 0:4, :], in_=xkD[:, 0:4, :])
            nc.sync.dma_start(out=xk_sb[:, 4:8, :], in_=xkD[:, 4:8, :])
            nc.sync.dma_start(out=m_sb, in_=mkD[:, :])
            nc.sync.dma_start(out=bv_sb, in_=_bcast_ap(bvD, [128, HL, 65]))
            nc.sync.dma_start(out=wv_sb, in_=wvD[:, :, :])
            for t0 in range(0, SKT, 2):
                t1 = min(t0 + 2, SKT)
                nc.sync.dma_start(
                    out=xv_sb[:, t0:t1, :, :], in_=xvD[:, t0:t1, :, :]
                )
            nc.sync.dma_start(out=wo_sb, in_=woD[:, :, :])

            # warm the Exp ACT table as soon as bq lands (~2us in)
            warm = tmp.tile([1, 1], f32, name="warm", tag="warm")
            nc.scalar.activation(
                out=warm[:, :], in_=bq_sb[0:1, 0:1],
                func=mybir.ActivationFunctionType.Exp,
            )

            # ---- phase 1: q/k projections (PSUM pj pool: 8 banks) ----
            with tc.tile_pool(name="pj", bufs=1, space="PSUM") as pj:
                for (which, slen, glist, xsb, wsb, bias_sb, dst) in (
                    ("q", S, [(o, 512) for o in range(0, S, 512)], xq_sb, wq_sb, bq_sb, qT_sb),
                    ("k", SK, kgroups, xk_sb, wk_sb, bk_sb, kT_sb),
                ):
                    pqt = {}
                    for ec in range(ECH):
                        xc = xsb[:, ec, :]
                        for pc in range(PC):
                            for gi, (go, gs) in enumerate(glist):
                                idx = pc * 4 + gi
                                if ec == 0:
                                    pqt[idx] = pj.tile(
                                        [128, 512], f32, name=f"pj{which}{idx}", tag=f"pj{idx}"
                                    )
                                nc.tensor.matmul(
                                    pqt[idx][:, 0:gs],
                                    wsb[:, ec, pc * 128 : pc * 128 + 128],
                                    xc[:, go : go + gs],
                                    start=(ec == 0),
                                    stop=(ec == ECH - 1),
                                )
                                if ec == ECH - 1:
                                    # bias add right after the group's last
                                    # matmul, alternating DVE/ACT
                                    if idx % 2 == 0:
                                        nc.vector.tensor_scalar_add(
                                            out=dst[pc][:, go : go + gs],
                                            in0=pqt[idx][:, 0:gs],
                                            scalar1=bias_sb[:, pc : pc + 1],
                                        )
                                    else:
                                        nc.scalar.activation(
                                            out=dst[pc][:, go : go + gs],
                                            in_=pqt[idx][:, 0:gs],
                                            func=mybir.ActivationFunctionType.Identity,
                                            bias=bias_sb[:, pc : pc + 1],
                                        )

            # ---- phase 2: lag-1 pipelined attention ----
            with (
                tc.tile_pool(name="scr", bufs=2, space="PSUM") as scr,
                tc.tile_pool(name="att", bufs=2, space="PSUM") as att,
                tc.tile_pool(name="rdr", bufs=2, space="DRAM") as rdr,
            ):
                probs = {}  # (stage, t) -> prb tile [128, 2, 512] (h2-major)

                def emit_scores_t(s, t):
                    g, c = s // 2, s % 2
                    gsl = slice(g * 512, (g + 1) * 512)
                    st = scr.tile([128, 2, 512], f32, name=f"sc{s}_{t}", tag="sc")
                    for h2 in range(2):
                        hsl = slice(h2 * 64, (h2 + 1) * 64)
                        nc.tensor.matmul(
                            st[:, h2, :],
                            kT_sb[c][hsl, t * 128 : (t + 1) * 128],
                            qT_sb[c][hsl, gsl],
                            start=True,
                            stop=True,
                        )
                    p_ = prb.tile([128, 2, 512], bf16, name=f"pb{s}_{t}", tag="pb")
                    nc.scalar.activation(
                        out=p_[:, :, :],
                        in_=st[:, :, :],
                        func=mybir.ActivationFunctionType.Exp,
                    )
                    probs[(s, t)] = p_

                def emit_attnv_t(s, t, at):
                    c = s % 2
                    p_ = probs.pop((s, t))
                    for h2 in range(2):
                        nc.tensor.matmul(
                            at[h2][:, :],
                            v_sb[t][:, 2 * c + h2, :],
                            p_[:, h2, :],
                            start=(t == 0),
                            stop=(t == SKT - 1),
                        )

                def emit_vproj_pass(t0):
                    tl = list(range(t0, min(t0 + 2, SKT)))
                    pvt = {}
                    for ec in range(ECH):
                        for j, t in enumerate(tl):
                            if ec == 0:
                                pvt[j] = att.tile(
                                    [128, EL], f32, name=f"pv{t}", tag=f"at{j}"
                                )
                            nc.tensor.matmul(
                                pvt[j][:, :],
                                xv_sb[:, t, ec, :],
                                wv_sb[:, ec, :],
                                start=(ec == 0),
                                stop=(ec == ECH - 1),
                            )
                    # epilogue on Pool: bias add, ones col, mask scale
                    for j, t in enumerate(tl):
                        pv_view = pvt[j][:, :].rearrange("p (h d) -> p h d", h=HL)
                        vt = vtp.tile([128, HL, 65], f32, name=f"vt{t}", tag="vtmp")
                        nc.vector.tensor_add(
                            out=vt[:, :, 0:64], in0=pv_view, in1=bv_sb[:, :, 0:64]
                        )
                        nc.vector.tensor_copy(
                            out=vt[:, :, 64:65], in_=bv_sb[:, :, 64:65]
                        )
                        nc.vector.tensor_scalar_mul(
                            out=v_sb[t][:, :, :],
                            in0=vt[:, :, :],
                            scalar1=m_sb[:, t : t + 1],
                        )

                def emit_norm(s, at):
                    g, c = s // 2, s % 2
                    gsl = slice(g * 512, (g + 1) * 512)
                    # approx-recip the denom rows (via SBUF), bcast via DRAM
                    rc = rcp.tile([1, 2, 512], f32, name=f"rc{s}", tag="rc")
                    nc.vector.tensor_copy(out=rc[:, 0, :], in_=at[0][64:65, :])
                    nc.vector.tensor_copy(out=rc[:, 1, :], in_=at[1][64:65, :])
                    rr = rcp.tile([1, 2, 512], f32, name=f"rr{s}", tag="rr")
                    nc.vector.reciprocal_approx_fast(
                        out=rr[0:1, :, :], in_=rc[0:1, :, :]
                    )
                    rd = rdr.tile([2, 512], f32, name=f"rd{s}", tag="rd")
                    nc.sync.dma_start(out=rd[:, :], in_=rr[0:1, :, :])
                    bs = bsb.tile([64, 2, 512], f32, name=f"bs{s}", tag="bs")
                    rdap = rd[:, :]
                    bc_ap = bass.AP(
                        tensor=rdap.tensor,
                        offset=rdap.offset,
                        ap=[[0, 64]] + [list(d) for d in rdap.ap],
                    )
                    nc.sync.dma_start(out=bs[:, :, :], in_=bc_ap)
                    nc.vector.tensor_mul(
                        out=aT_sb[c][0:64, gsl], in0=at[0][0:64, :], in1=bs[:, 0, :]
                    )
                    tb = tmp.tile([64, 512], bf16, name=f"tb{s}", tag="tb")
                    nc.vector.tensor_mul(out=tb, in0=at[1][0:64, :], in1=bs[:, 1, :])
                    nc.sync.dma_start(out=aT_sb[c][64:128, gsl], in_=tb)

                def new_at(s):
                    return [
                        att.tile([65, 512], f32, name=f"at{h2}_{s}", tag=f"at{h2}")
                        for h2 in range(2)
                    ]

                def emit_outproj(sl, cs=(0, 1), pots=None, drain=False):
                    ssl = slice(sl * 128, (sl + 1) * 128)
                    if pots is None:
                        pots = scr.tile([128, 2, 512], f32, name=f"po{sl}", tag="sc")
                    for c in cs:
                        for jg in range(2):
                            nc.tensor.matmul(
                                pots[:, jg, :],
                                aT_sb[c][:, ssl],
                                wo_sb[:, c, jg * 512 : (jg + 1) * 512],
                                start=(c == 0),
                                stop=(c == PC - 1),
                            )
                    if cs[-1] != PC - 1:
                        return pots
                    ot = osb.tile([128, E], bf16, name=f"ot{sl}", tag="ot")
                    ov = ot[:, :].rearrange("p (j e) -> p j e", j=2)
                    # PSUM->SBUF copies on DVE; in the drain ACT is free so
                    # split the halves across engines
                    nc.vector.tensor_copy(out=ov[:, 0:1, :], in_=pots[:, 0:1, :])
                    if drain:
                        nc.scalar.activation(
                            out=ov[:, 1:2, :], in_=pots[:, 1:2, :],
                            func=mybir.ActivationFunctionType.Identity,
                        )
                        nc.scalar.dma_start(out=out[ssl, :], in_=ot)
                    else:
                        nc.vector.tensor_copy(out=ov[:, 1:2, :], in_=pots[:, 1:2, :])
                        nc.sync.dma_start(out=out[ssl, :], in_=ot)
                    return None

                NST = 2 * SGRP
                at_cur = None     # accumulating at tiles for stage s-1
                next_sl = 0
                for s in range(NST):
                    at_prev = at_cur
                    at_cur = new_at(s - 1) if s >= 2 else None
                    # out-proj slots this stage: group gp=(s-3)//2... emit when
                    # group (sl//4) was normed >= 1 stage ago
                    for t in range(SKT):
                        # attnV first: it never waits on this stage's exp, so
                        # the PE stays busy while ACT catches up
                        if s >= 2:
                            emit_attnv_t(s - 1, t, at_cur)
                        emit_scores_t(s, t)
                        if s == 0 and t in (5, 7):
                            emit_vproj_pass(t - 5)
                        elif s == 1 and t % 2 == 0 and t >= 4:
                            emit_vproj_pass(t)
                        if (
                            s >= 3
                            and t in (2, 5)
                            and next_sl < 10
                            and next_sl // 4 <= (s - 3) // 2
                        ):
                            emit_outproj(next_sl)
                            next_sl += 1
                    if s == 1:
                        at_cur = new_at(0)
                        for t in range(SKT):
                            emit_attnv_t(0, t, at_cur)
                    if at_cur is not None:
                        emit_norm(s - 1, at_cur)

                # ---- drain: attnV(7) + remaining out-proj, c-split tail ----
                at_last = new_at(NST - 1)
                for t in range(SKT):
                    emit_attnv_t(NST - 1, t, at_last)
                    if t == 1:
                        emit_outproj(next_sl)  # sl 10
                        next_sl += 1
                    if t == 3:
                        emit_outproj(next_sl, drain=True)  # sl 11
                        next_sl += 1
                # g3 c=0 parts (aT[0] ready via norm(6)); sc banks free
                pot_a = emit_outproj(12, cs=(0,))
                pot_b = emit_outproj(13, cs=(0,))
                emit_norm(NST - 1, at_last)
                # at banks free after norm reads; run sl14/15 c0 there
                pot_c = [
                    att.tile([128, 1, 512], f32, name=f"poc{j}", tag=f"at{j}")
                    for j in range(2)
                ]
                pot_d = [
                    att.tile([128, 1, 512], f32, name=f"pod{j}", tag=f"at{j}")
                    for j in range(2)
                ]
                for jg in range(2):
                    nc.tensor.matmul(
                        pot_c[jg][:, 0, :],
                        aT_sb[0][:, 14 * 128 : 15 * 128],
                        wo_sb[:, 0, jg * 512 : (jg + 1) * 512],
                        start=True, stop=False,
                    )
                    nc.tensor.matmul(
                        pot_d[jg][:, 0, :],
                        aT_sb[0][:, 15 * 128 : 16 * 128],
                        wo_sb[:, 0, jg * 512 : (jg + 1) * 512],
                        start=True, stop=False,
                    )
                emit_outproj(12, cs=(1,), pots=pot_a, drain=True)
                emit_outproj(13, cs=(1,), pots=pot_b, drain=True)
                for sl, pot in ((14, pot_c), (15, pot_d)):
                    ssl = slice(sl * 128, (sl + 1) * 128)
                    ot = osb.tile([128, E], bf16, name=f"otd{sl}", tag="ot")
                    ov = ot[:, :].rearrange("p (j e) -> p j e", j=2)
                    for jg in range(2):
                        nc.tensor.matmul(
                            pot[jg][:, 0, :],
                            aT_sb[1][:, ssl],
                            wo_sb[:, 1, jg * 512 : (jg + 1) * 512],
                            start=False, stop=True,
                        )
                        # copy each half right after its matmul; ACT is free
                        # in the drain, so split DVE/ACT
                        if jg == 0:
                            nc.vector.tensor_copy(
                                out=ov[:, 0:1, :], in_=pot[0][:, :, :]
                            )
                        else:
                            nc.scalar.activation(
                                out=ov[:, 1:2, :], in_=pot[1][:, :, :],
                                func=mybir.ActivationFunctionType.Identity,
                            )
                    nc.scalar.dma_start(out=out[ssl, :], in_=ot)

    nc.finalize()
    return nc


def _get(SK: int):
    if SK not in _cache:
        _cache[SK] = _build(SK)
    return _cache[SK]


def _part_major(a2d: np.ndarray, nch: int) -> np.ndarray:
    """[nch*128, M] -> [128, nch, M] partition-major contiguous bf16."""
    M = a2d.shape[1]
    return np.ascontiguousarray(
        a2d.reshape(nch, 128, M).transpose(1, 0, 2)
    ).astype(nbf16)


def kernel(**inputs) -> np.ndarray:
    global LAST_EXEC_NS, LAST_RESULTS

    q = np.asarray(inputs["query"], dtype=np.float32)
    k = np.asarray(inputs["key"], dtype=np.float32)
    v = np.asarray(inputs["value"], dtype=np.float32)
    kpm = np.asarray(inputs["key_padding_mask"]).astype(bool)
    Wq = np.asarray(inputs["Wq"], dtype=np.float32)
    bq = np.asarray(inputs["bq"], dtype=np.float32)
    Wk = np.asarray(inputs["Wk"], dtype=np.float32)
    bk = np.asarray(inputs["bk"], dtype=np.float32)
    Wv = np.asarray(inputs["Wv"], dtype=np.float32)
    bv = np.asarray(inputs["bv"], dtype=np.float32)
    Wo = np.asarray(inputs["Wo"], dtype=np.float32)
    bo = np.asarray(inputs["bo"], dtype=np.float32)

    compact = not os.environ.get("KERNEL_NO_COMPACT")
    if compact:
        valid = [np.nonzero(~kpm[b])[0] for b in range(B)]
        nv = max(len(ix) for ix in valid)
        SK = max(128, ((nv + 127) // 128) * 128)
        if SK > S:
            SK = S
            compact = False
    if not compact:
        SK = S
        valid = [np.arange(S) for _ in range(B)]
    SKT = SK // 128

    nc = _get(SK)

    per_b = []
    for b in range(B):
        ix = valid[b]
        n = len(ix)
        xq = _part_major(q[b].T, ECH)                      # [128, ECH, S]
        kc = np.zeros((SK, E), dtype=np.float32)
        vc = np.zeros((SK, E), dtype=np.float32)
        kc[:n] = k[b][ix]
        vc[:n] = v[b][ix]
        xk = _part_major(kc.T, ECH)
        # [128, SKT, ECH, 128] k-tile-major
        xv = np.ascontiguousarray(
            vc.T.reshape(ECH, 128, SKT, 128).transpose(1, 2, 0, 3)
        ).astype(nbf16)
        mv = np.zeros(SK, dtype=np.float32)
        if compact:
            mv[:n] = 1.0
        else:
            mv[:] = (~kpm[b]).astype(np.float32)
        m2 = np.ascontiguousarray(mv.reshape(SKT, 128).T)  # [128, SKT]
        per_b.append((xq, xk, xv, m2))

    in_maps = []
    for cid in range(N_CORES):
        b, hg = cid // 4, cid % 4
        hsl = slice(hg * EL, (hg + 1) * EL)
        xq, xk, xv, m2 = per_b[b]
        bvh = bv[hsl].reshape(HL, 64)
        bvA = np.concatenate([bvh, np.ones((HL, 1), np.float32)], axis=1).ravel()
        in_maps.append(
            {
                "xqD": xq,
                "xkD": xk,
                "xvD": xv,
                "wqD": _part_major((Wq[hsl] / 8.0).T, ECH),
                "wkD": _part_major(Wk[hsl].T, ECH),
                "wvD": _part_major(Wv[hsl].T, ECH),
                "woD": _part_major(Wo[:, hsl].T, PC),
                "bqD": np.ascontiguousarray((bq[hsl] / 8.0).reshape(PC, 128).T),
                "bkD": np.ascontiguousarray(bk[hsl].reshape(PC, 128).T),
                "bvD": bvA,
                "mkD": m2,
            }
        )

    trace = bool(os.environ.get("KERNEL_TRACE"))
    res = run_bass_kernel_spmd(
        nc, in_maps, core_ids=list(range(N_CORES)), trace=trace
    )
    LAST_EXEC_NS = res.exec_time_ns
    LAST_RESULTS = res

    out = np.empty((B, S, E), dtype=np.float32)
    for b in range(B):
        acc = res.results[b * 4]["out"].astype(np.float32)
        for hg in range(1, 4):
            acc = acc + res.results[b * 4 + hg]["out"].astype(np.float32)
        out[b] = acc + bo
    return out


# revision 29
# speedup vs baseline: 1.1711x; 1.1711x over previous
"""Multihead attention on 8 Trainium2 cores (Bass/Tile).

Sharding: core = (batch b, head-group hg); 2 batches x 4 head-groups,
4 heads per core (head dim 64, local width 256).

Per core (matmul operands bf16, PSUM accumulation fp32):
  qT = (Wq[hg]/8 @ x_q^T)          [256, 2048]   (e' on partitions)
  kT = (Wk[hg]  @ x_k^T)           [256, SK]
  v  = (x_k  @ Wv[hg]^T)           [SK, 256]     (+ ones col -> denominator)
  scoresT[sk, sq] = kT^T-slices x qT  (PE, per k-tile, both head-halves)
  probsT = exp(scoresT)            (ACT, PSUM fp32 -> SBUF bf16)
  attnT[d, sq], denom[sq] = v_aug^T @ probsT
  attn = attnT * recip(denom)      (DVE recip on the PSUM denom row,
                                    DRAM-bounce broadcast, DVE muls)
  out_partial[s, :] = attn^T-chunks x Wo[:, hg]^T   (no bias on device)
Host: out[b] = sum of the 4 head-group partials + bo.

Schedule: lag-1 software pipeline over 8 stages (4 q-groups x 2
partition-chunks). Stage s emits scores(s)+exp(s) per k-tile with the
scores PSUM double-buffered via tag rotation, attnV(s-1) interleaved
per k-tile, norm(s-1) at stage end, and out-projection tiles filled
2/stage. The v-projection runs inside stage 1. The drain c-splits the
last out-proj tiles around the final norm chain so the PE never idles.

DMA: all host tensors are pre-arranged so every SBUF partition row is
one contiguous descriptor (128 descriptors per chunk DMA); wq/xq
chunks are issued first and gate the first projection matmuls ~2us in.

Mask handling: the key_padding_mask (and compaction padding) zeroes v
rows AND the ones-column, so masked keys contribute exactly 0 to both
the numerator and the softmax denominator.

Compaction: only valid (unmasked) key positions are shipped per batch,
padded to a multiple of 128. Kernels are compiled once per SK.
"""

import os
import sys

sys.path.insert(0, "/opt/trn_rl_repo")

import ml_dtypes
import numpy as np

import concourse.bass as bass
import concourse.mybir as mybir
import concourse.tile as tile
from concourse import bacc
from concourse.bass_utils import run_bass_kernel_spmd

B, S, E, H, D = 2, 2048, 1024, 16, 64
N_CORES = 8
HL = H // 4          # 4 heads per core
EL = HL * D          # 256 local embed width
PC = EL // 128       # 2 partition chunks of local heads
ECH = E // 128       # 8 contraction chunks for projections
SGRP = S // 512      # 4 query groups
SQT = S // 128       # 16 query tiles

f32 = mybir.dt.float32
bf16 = mybir.dt.bfloat16
nbf16 = ml_dtypes.bfloat16

_cache: dict[int, object] = {}
LAST_EXEC_NS = None
LAST_RESULTS = None


def _bcast_ap(handle, shape):
    """DRAM AP broadcast along partitions: shape [128, ...dims of handle]."""
    ap = handle[:]
    dims = [[0, shape[0]]]
    sizes = shape[1:]
    stride = 1
    rev = []
    for s in reversed(sizes):
        rev.append([stride, s])
        stride *= s
    dims.extend(reversed(rev))
    return bass.AP(tensor=ap.tensor, offset=0, ap=dims)


def _build(SK: int):
    SKT = SK // 128
    kgroups = [(o, min(512, SK - o)) for o in range(0, SK, 512)]

    nc = bacc.Bacc(None, target_bir_lowering=False)

    # all host-side pre-arranged: partition-major, contiguous per partition
    xqD = nc.dram_tensor("xqD", [128, ECH, S], bf16, kind="ExternalInput")
    xkD = nc.dram_tensor("xkD", [128, ECH, SK], bf16, kind="ExternalInput")
    # xv is k-tile-major so v-proj passes gate on per-tile chunks
    xvD = nc.dram_tensor("xvD", [128, SKT, ECH, 128], bf16, kind="ExternalInput")
    wqD = nc.dram_tensor("wqD", [128, ECH, EL], bf16, kind="ExternalInput")
    wkD = nc.dram_tensor("wkD", [128, ECH, EL], bf16, kind="ExternalInput")
    wvD = nc.dram_tensor("wvD", [128, ECH, EL], bf16, kind="ExternalInput")
    woD = nc.dram_tensor("woD", [128, PC, E], bf16, kind="ExternalInput")
    bqD = nc.dram_tensor("bqD", [128, PC], f32, kind="ExternalInput")
    bkD = nc.dram_tensor("bkD", [128, PC], f32, kind="ExternalInput")
    bvD = nc.dram_tensor("bvD", [HL * 65], f32, kind="ExternalInput")
    mkD = nc.dram_tensor("mkD", [128, SKT], f32, kind="ExternalInput")
    out = nc.dram_tensor("out", [S, E], bf16, kind="ExternalOutput")

    with tile.TileContext(nc) as tc, nc.allow_low_precision("bf16 attention"):
        with (
            tc.tile_pool(name="persist", bufs=1) as pp,
            tc.tile_pool(name="prb", bufs=2 * SKT + 2) as prb,
            tc.tile_pool(name="rcp", bufs=2) as rcp,
            tc.tile_pool(name="bsb", bufs=2) as bsb,
            tc.tile_pool(name="tmp", bufs=2) as tmp,
            tc.tile_pool(name="vtp", bufs=2) as vtp,
            tc.tile_pool(name="osb", bufs=3) as osb,
        ):
            # ---- persistent tiles ----
            wq_sb = pp.tile([128, ECH, EL], bf16, name="wq_sb", tag="wq_sb")
            wk_sb = pp.tile([128, ECH, EL], bf16, name="wk_sb", tag="wk_sb")
            wv_sb = pp.tile([128, ECH, EL], bf16, name="wv_sb", tag="wv_sb")
            wo_sb = pp.tile([128, PC, E], bf16, name="wo_sb", tag="wo_sb")
            bq_sb = pp.tile([128, PC], f32, name="bq_sb", tag="bq_sb")
            bk_sb = pp.tile([128, PC], f32, name="bk_sb", tag="bk_sb")
            bv_sb = pp.tile([128, HL, 65], f32, name="bv_sb", tag="bv_sb")
            m_sb = pp.tile([128, SKT], f32, name="m_sb", tag="m_sb")
            xq_sb = pp.tile([128, ECH, S], bf16, name="xq_sb", tag="xq_sb")
            xk_sb = pp.tile([128, ECH, SK], bf16, name="xk_sb", tag="xk_sb")
            xv_sb = pp.tile([128, SKT, ECH, 128], bf16, name="xv_sb", tag="xv_sb")

            qT_sb = [pp.tile([128, S], bf16, name=f"qT{c}", tag=f"qT{c}") for c in range(PC)]
            kT_sb = [pp.tile([128, SK], bf16, name=f"kT{c}", tag=f"kT{c}") for c in range(PC)]
            aT_sb = [pp.tile([128, S], bf16, name=f"aT{c}", tag=f"aT{c}") for c in range(PC)]
            v_sb = [
                pp.tile([128, HL, 65], bf16, name=f"v{t}", tag=f"v{t}") for t in range(SKT)
            ]

            # ---- DMA issue order == consumption order; each dma_start
            # costs ~0.7us serial on the SP sequencer, so keep the count low
            # while still chunking enough to gate the projection pipeline.
            nc.sync.dma_start(out=wq_sb, in_=wqD[:, :, :])
            nc.sync.dma_start(out=xq_sb[:, 0, :], in_=xqD[:, 0, :])
            nc.sync.dma_start(out=xq_sb[:, 1, :], in_=xqD[:, 1, :])
            nc.sync.dma_start(out=xq_sb[:, 2:4, :], in_=xqD[:, 2:4, :])
            nc.sync.dma_start(out=bq_sb, in_=bqD[:, :])
            nc.sync.dma_start(out=bk_sb, in_=bkD[:, :])
            nc.sync.dma_start(out=xq_sb[:, 4:8, :], in_=xqD[:, 4:8, :])
            nc.sync.dma_start(out=wk_sb, in_=wkD[:, :, :])
            nc.sync.dma_start(out=xk_sb[:, 0:4, :], in_=xkD[:, 0:4, :])
            nc.sync.dma_start(out=xk_sb[:, 4:8, :], in_=xkD[:, 4:8, :])
            nc.sync.dma_start(out=m_sb, in_=mkD[:, :])
            nc.sync.dma_start(out=bv_sb, in_=_bcast_ap(bvD, [128, HL, 65]))
            nc.sync.dma_start(out=wv_sb, in_=wvD[:, :, :])
            for t0 in range(0, SKT, 2):
                t1 = min(t0 + 2, SKT)
                nc.sync.dma_start(
                    out=xv_sb[:, t0:t1, :, :], in_=xvD[:, t0:t1, :, :]
                )
            nc.sync.dma_start(out=wo_sb, in_=woD[:, :, :])

            # warm the Exp ACT table as soon as bq lands (~2us in)
            warm = tmp.tile([1, 1], f32, name="warm", tag="warm")
            nc.scalar.activation(
                out=warm[:, :], in_=bq_sb[0:1, 0:1],
                func=mybir.ActivationFunctionType.Exp,
            )

            # ---- phase 1: q/k projections (PSUM pj pool: 8 banks) ----
            with tc.tile_pool(name="pj", bufs=1, space="PSUM") as pj:
                for (which, slen, glist, xsb, wsb, bias_sb, dst) in (
                    ("q", S, [(o, 512) for o in range(0, S, 512)], xq_sb, wq_sb, bq_sb, qT_sb),
                    ("k", SK, kgroups, xk_sb, wk_sb, bk_sb, kT_sb),
                ):
                    pqt = {}
                    for ec in range(ECH):
                        xc = xsb[:, ec, :]
                        for pc in range(PC):
                            for gi, (go, gs) in enumerate(glist):
                                idx = pc * 4 + gi
                                if ec == 0:
                                    pqt[idx] = pj.tile(
                                        [128, 512], f32, name=f"pj{which}{idx}", tag=f"pj{idx}"
                                    )
                                nc.tensor.matmul(
                                    pqt[idx][:, 0:gs],
                                    wsb[:, ec, pc * 128 : pc * 128 + 128],
                                    xc[:, go : go + gs],
                                    start=(ec == 0),
                                    stop=(ec == ECH - 1),
                                )
                                if ec == ECH - 1:
                                    # bias add right after the group's last
                                    # matmul, alternating DVE/ACT
                                    if idx % 2 == 0:
                                        nc.vector.tensor_scalar_add(
                                            out=dst[pc][:, go : go + gs],
                                            in0=pqt[idx][:, 0:gs],
                                            scalar1=bias_sb[:, pc : pc + 1],
                                        )
                                    else:
                                        nc.scalar.activation(
                                            out=dst[pc][:, go : go + gs],
                                            in_=pqt[idx][:, 0:gs],
                                            func=mybir.ActivationFunctionType.Identity,
                                            bias=bias_sb[:, pc : pc + 1],
                                        )

            # ---- phase 2: lag-1 pipelined attention ----
            with (
                tc.tile_pool(name="scr", bufs=2, space="PSUM") as scr,
                tc.tile_pool(name="att", bufs=2, space="PSUM") as att,
                tc.tile_pool(name="rdr", bufs=2, space="DRAM") as rdr,
            ):
                probs = {}  # (stage, t) -> prb tile [128, 2, 512] (h2-major)

                def emit_scores_t(s, t):
                    g, c = s // 2, s % 2
                    gsl = slice(g * 512, (g + 1) * 512)
                    st = scr.tile([128, 2, 512], f32, name=f"sc{s}_{t}", tag="sc")
                    for h2 in range(2):
                        hsl = slice(h2 * 64, (h2 + 1) * 64)
                        nc.tensor.matmul(
                            st[:, h2, :],
                            kT_sb[c][hsl, t * 128 : (t + 1) * 128],
                            qT_sb[c][hsl, gsl],
                            start=True,
                            stop=True,
                        )
                    p_ = prb.tile([128, 2, 512], bf16, name=f"pb{s}_{t}", tag="pb")
                    nc.scalar.activation(
                        out=p_[:, :, :],
                        in_=st[:, :, :],
                        func=mybir.ActivationFunctionType.Exp,
                    )
                    probs[(s, t)] = p_

                def emit_attnv_t(s, t, at):
                    c = s % 2
                    p_ = probs.pop((s, t))
                    for h2 in range(2):
                        nc.tensor.matmul(
                            at[h2][:, :],
                            v_sb[t][:, 2 * c + h2, :],
                            p_[:, h2, :],
                            start=(t == 0),
                            stop=(t == SKT - 1),
                        )

                def emit_vproj_pass(t0):
                    tl = list(range(t0, min(t0 + 2, SKT)))
                    pvt = {}
                    for ec in range(ECH):
                        for j, t in enumerate(tl):
                            if ec == 0:
                                pvt[j] = att.tile(
                                    [128, EL], f32, name=f"pv{t}", tag=f"at{j}"
                                )
                            nc.tensor.matmul(
                                pvt[j][:, :],
                                xv_sb[:, t, ec, :],
                                wv_sb[:, ec, :],
                                start=(ec == 0),
                                stop=(ec == ECH - 1),
                            )
                    # epilogue on Pool: bias add, ones col, mask scale
                    for j, t in enumerate(tl):
                        pv_view = pvt[j][:, :].rearrange("p (h d) -> p h d", h=HL)
                        vt = vtp.tile([128, HL, 65], f32, name=f"vt{t}", tag="vtmp")
                        nc.vector.tensor_add(
                            out=vt[:, :, 0:64], in0=pv_view, in1=bv_sb[:, :, 0:64]
                        )
                        nc.vector.tensor_copy(
                            out=vt[:, :, 64:65], in_=bv_sb[:, :, 64:65]
                        )
                        nc.vector.tensor_scalar_mul(
                            out=v_sb[t][:, :, :],
                            in0=vt[:, :, :],
                            scalar1=m_sb[:, t : t + 1],
                        )

                def emit_norm(s, at):
                    g, c = s // 2, s % 2
                    gsl = slice(g * 512, (g + 1) * 512)
                    # approx-recip the denom rows (via SBUF), bcast via DRAM
                    rc = rcp.tile([1, 2, 512], f32, name=f"rc{s}", tag="rc")
                    nc.vector.tensor_copy(out=rc[:, 0, :], in_=at[0][64:65, :])
                    nc.vector.tensor_copy(out=rc[:, 1, :], in_=at[1][64:65, :])
                    rr = rcp.tile([1, 2, 512], f32, name=f"rr{s}", tag="rr")
                    nc.vector.reciprocal_approx_fast(
                        out=rr[0:1, :, :], in_=rc[0:1, :, :]
                    )
                    rd = rdr.tile([2, 512], f32, name=f"rd{s}", tag="rd")
                    nc.sync.dma_start(out=rd[:, :], in_=rr[0:1, :, :])
                    bs = bsb.tile([64, 2, 512], f32, name=f"bs{s}", tag="bs")
                    rdap = rd[:, :]
                    bc_ap = bass.AP(
                        tensor=rdap.tensor,
                        offset=rdap.offset,
                        ap=[[0, 64]] + [list(d) for d in rdap.ap],
                    )
                    nc.sync.dma_start(out=bs[:, :, :], in_=bc_ap)
                    nc.vector.tensor_mul(
                        out=aT_sb[c][0:64, gsl], in0=at[0][0:64, :], in1=bs[:, 0, :]
                    )
                    tb = tmp.tile([64, 512], bf16, name=f"tb{s}", tag="tb")
                    nc.vector.tensor_mul(out=tb, in0=at[1][0:64, :], in1=bs[:, 1, :])
                    nc.sync.dma_start(out=aT_sb[c][64:128, gsl], in_=tb)

                def new_at(s):
                    return [
                        att.tile([65, 512], f32, name=f"at{h2}_{s}", tag=f"at{h2}")
                        for h2 in range(2)
                    ]

                def emit_outproj(sl, cs=(0, 1), pots=None, drain=False):
                    ssl = slice(sl * 128, (sl + 1) * 128)
                    if pots is None:
                        pots = scr.tile([128, 2, 512], f32, name=f"po{sl}", tag="sc")
                    for c in cs:
                        for jg in range(2):
                            nc.tensor.matmul(
                                pots[:, jg, :],
                                aT_sb[c][:, ssl],
                                wo_sb[:, c, jg * 512 : (jg + 1) * 512],
                                start=(c == 0),
                                stop=(c == PC - 1),
                            )
                    if cs[-1] != PC - 1:
                        return pots
                    ot = osb.tile([128, E], bf16, name=f"ot{sl}", tag="ot")
                    ov = ot[:, :].rearrange("p (j e) -> p j e", j=2)
                    # PSUM->SBUF copies on DVE; in the drain ACT is free so
                    # split the halves across engines
                    nc.vector.tensor_copy(out=ov[:, 0:1, :], in_=pots[:, 0:1, :])
                    if drain:
                        nc.scalar.activation(
                            out=ov[:, 1:2, :], in_=pots[:, 1:2, :],
                            func=mybir.ActivationFunctionType.Identity,
                        )
                        nc.scalar.dma_start(out=out[ssl, :], in_=ot)
                    else:
                        nc.vector.tensor_copy(out=ov[:, 1:2, :], in_=pots[:, 1:2, :])
                        nc.sync.dma_start(out=out[ssl, :], in_=ot)
                    return None

                NST = 2 * SGRP
                at_cur = None     # accumulating at tiles for stage s-1
                next_sl = 0
                for s in range(NST):
                    at_prev = at_cur
                    at_cur = new_at(s - 1) if s >= 2 else None
                    # out-proj slots this stage: group gp=(s-3)//2... emit when
                    # group (sl//4) was normed >= 1 stage ago
                    for t in range(SKT):
                        emit_scores_t(s, t)
                        if s == 0 and t in (5, 7):
                            emit_vproj_pass(t - 5)
                        elif s == 1 and t % 2 == 0 and t >= 4:
                            emit_vproj_pass(t)
                        elif s >= 2:
                            emit_attnv_t(s - 1, t, at_cur)
                        if (
                            s >= 3
                            and t in (2, 5)
                            and next_sl < 10
                            and next_sl // 4 <= (s - 3) // 2
                        ):
                            emit_outproj(next_sl)
                            next_sl += 1
                    if s == 1:
                        at_cur = new_at(0)
                        for t in range(SKT):
                            emit_attnv_t(0, t, at_cur)
                    if at_cur is not None:
                        emit_norm(s - 1, at_cur)

                # ---- drain: attnV(7) + remaining out-proj, c-split tail ----
                at_last = new_at(NST - 1)
                for t in range(SKT):
                    emit_attnv_t(NST - 1, t, at_last)
                    if t == 1:
                        emit_outproj(next_sl)  # sl 10
                        next_sl += 1
                    if t == 3:
                        emit_outproj(next_sl, drain=True)  # sl 11
                        next_sl += 1
                # g3 c=0 parts (aT[0] ready via norm(6)); sc banks free
                pot_a = emit_outproj(12, cs=(0,))
                pot_b = emit_outproj(13, cs=(0,))
                emit_norm(NST - 1, at_last)
                # at banks free after norm reads; run sl14/15 c0 there
                pot_c = [
                    att.tile([128, 1, 512], f32, name=f"poc{j}", tag=f"at{j}")
                    for j in range(2)
                ]
                pot_d = [
                    att.tile([128, 1, 512], f32, name=f"pod{j}", tag=f"at{j}")
                    for j in range(2)
                ]
                for jg in range(2):
                    nc.tensor.matmul(
                        pot_c[jg][:, 0, :],
                        aT_sb[0][:, 14 * 128 : 15 * 128],
                        wo_sb[:, 0, jg * 512 : (jg + 1) * 512],
                        start=True, stop=False,
                    )
                    nc.tensor.matmul(
                        pot_d[jg][:, 0, :],
                        aT_sb[0][:, 15 * 128 : 16 * 128],
                        wo_sb[:, 0, jg * 512 : (jg + 1) * 512],
                        start=True, stop=False,
                    )
                emit_outproj(12, cs=(1,), pots=pot_a, drain=True)
                emit_outproj(13, cs=(1,), pots=pot_b, drain=True)
                for sl, pot in ((14, pot_c), (15, pot_d)):
                    ssl = slice(sl * 128, (sl + 1) * 128)
                    ot = osb.tile([128, E], bf16, name=f"otd{sl}", tag="ot")
                    ov = ot[:, :].rearrange("p (j e) -> p j e", j=2)
                    for jg in range(2):
                        nc.tensor.matmul(
                            pot[jg][:, 0, :],
                            aT_sb[1][:, ssl],
                            wo_sb[:, 1, jg * 512 : (jg + 1) * 512],
                            start=False, stop=True,
                        )
                        # copy each half right after its matmul; ACT is free
                        # in the drain, so split DVE/ACT
                        if jg == 0:
                            nc.vector.tensor_copy(
                                out=ov[:, 0:1, :], in_=pot[0][:, :, :]
                            )
                        else:
                            nc.scalar.activation(
                                out=ov[:, 1:2, :], in_=pot[1][:, :, :],
                                func=mybir.ActivationFunctionType.Identity,
                            )
                    nc.scalar.dma_start(out=out[ssl, :], in_=ot)

    nc.finalize()
    return nc


def _get(SK: int):
    if SK not in _cache:
        _cache[SK] = _build(SK)
    return _cache[SK]


def _part_major(a2d: np.ndarray, nch: int) -> np.ndarray:
    """[nch*128, M] -> [128, nch, M] partition-major contiguous bf16."""
    M = a2d.shape[1]
    return np.ascontiguousarray(
        a2d.reshape(nch, 128, M).transpose(1, 0, 2)
    ).astype(nbf16)


def kernel(**inputs) -> np.ndarray:
    global LAST_EXEC_NS, LAST_RESULTS

    q = np.asarray(inputs["query"], dtype=np.float32)
    k = np.asarray(inputs["key"], dtype=np.float32)
    v = np.asarray(inputs["value"], dtype=np.float32)
    kpm = np.asarray(inputs["key_padding_mask"]).astype(bool)
    Wq = np.asarray(inputs["Wq"], dtype=np.float32)
    bq = np.asarray(inputs["bq"], dtype=np.float32)
    Wk = np.asarray(inputs["Wk"], dtype=np.float32)
    bk = np.asarray(inputs["bk"], dtype=np.float32)
    Wv = np.asarray(inputs["Wv"], dtype=np.float32)
    bv = np.asarray(inputs["bv"], dtype=np.float32)
    Wo = np.asarray(inputs["Wo"], dtype=np.float32)
    bo = np.asarray(inputs["bo"], dtype=np.float32)

    compact = not os.environ.get("KERNEL_NO_COMPACT")
    if compact:
        valid = [np.nonzero(~kpm[b])[0] for b in range(B)]
        nv = max(len(ix) for ix in valid)
        SK = max(128, ((nv + 127) // 128) * 128)
        if SK > S:
            SK = S
            compact = False
    if not compact:
        SK = S
        valid = [np.arange(S) for _ in range(B)]
    SKT = SK // 128

    nc = _get(SK)

    per_b = []
    for b in range(B):
        ix = valid[b]
        n = len(ix)
        xq = _part_major(q[b].T, ECH)                      # [128, ECH, S]
        kc = np.zeros((SK, E), dtype=np.float32)
        vc = np.zeros((SK, E), dtype=np.float32)
        kc[:n] = k[b][ix]
        vc[:n] = v[b][ix]
        xk = _part_major(kc.T, ECH)
        # [128, SKT, ECH, 128] k-tile-major
        xv = np.ascontiguousarray(
            vc.T.reshape(ECH, 128, SKT, 128).transpose(1, 2, 0, 3)
        ).astype(nbf16)
        mv = np.zeros(SK, dtype=np.float32)
        if compact:
            mv[:n] = 1.0
        else:
            mv[:] = (~kpm[b]).astype(np.float32)
        m2 = np.ascontiguousarray(mv.reshape(SKT, 128).T)  # [128, SKT]
        per_b.append((xq, xk, xv, m2))

    in_maps = []
    for cid in range(N_CORES):
        b, hg = cid // 4, cid % 4
        hsl = slice(hg * EL, (hg + 1) * EL)
        xq, xk, xv, m2 = per_b[b]
        bvh = bv[hsl].reshape(HL, 64)
        bvA = np.concatenate([bvh, np.ones((HL, 1), np.float32)], axis=1).ravel()
        in_maps.append(
            {
                "xqD": xq,
                "xkD": xk,
                "xvD": xv,
                "wqD": _part_major((Wq[hsl] / 8.0).T, ECH),
                "wkD": _part_major(Wk[hsl].T, ECH),
                "wvD": _part_major(Wv[hsl].T, ECH),
                "woD": _part_major(Wo[:, hsl].T, PC),
                "bqD": np.ascontiguousarray((bq[hsl] / 8.0).reshape(PC, 128).T),
                "bkD": np.ascontiguousarray(bk[hsl].reshape(PC, 128).T),
                "bvD": bvA,
                "mkD": m2,
            }
        )

    trace = bool(os.environ.get("KERNEL_TRACE"))
    res = run_bass_kernel_spmd(
        nc, in_maps, core_ids=list(range(N_CORES)), trace=trace
    )
    LAST_EXEC_NS = res.exec_time_ns
    LAST_RESULTS = res

    out = np.empty((B, S, E), dtype=np.float32)
    for b in range(B):
        acc = res.results[b * 4]["out"].astype(np.float32)
        for hg in range(1, 4):
            acc = acc + res.results[b * 4 + hg]["out"].astype(np.float32)
        out[b] = acc + bo
    return out
